# revision 17
# baseline (speedup 1.0000x reference)
"""Trainium2 Bass kernel for grouped per-atom MLPs (AtomicNN energy eval).

Math: e[s, a] = W3[a].T tanh(W2[a].T tanh(W1[a].T g[s,a] + b1[a]) + b2[a]) + b3[a]
Shapes: g [4096, 1024, 5], per-atom MLP 5 -> 64 -> 64 -> 1.

Strategy (8 NeuronCores, SPMD, atom-sharded: core c owns atoms [128c, 128c+128)):
 - Atoms processed in pairs (2x64 = 128 PE rows/cols); 4096 structs stream
   through in 8 chunks of NS=512 (one PSUM bank).
 - Layer 1 (K=11 incl. a ones-row folding b1): pairs grouped in TRIOS and
   row-tiled via tile_position=(32j, 0) -- 3 concurrent matmuls in different
   32-row groups of the PE array (~3x layer-1 throughput).
 - Layer 2: full-array matmuls (K=128, M=128), pairs processed in duos so a
   single [128, 1024] PSUM tile feeds one batched tanh.
 - Layer 3: 64 accumulating matmuls per chunk into a single PSUM bank:
   lhsT[:, 2p] = [W3[2p]; 0], lhsT[:, 2p+1] = [0; W3[2p+1]] builds the
   [128 atoms, 512 structs] transposed-output block directly.
 - PSUM budget: z1 trio (3 banks) + z2 duo x2 (4) + et (1) = 8 banks exactly.
 - The PE runs throttled (1.2 GHz) unless an activity window is gap-free;
   with tanh offloaded the PE stream is the dense bottleneck and the HAM
   un-throttles for long stretches.
 - tanh is the critical path (ScalarE is the only tanh engine at 1 elem/
   lane/cycle): ~30% of tanh units are offloaded to the Vector engine via two
   registered custom-DVE ops implementing clip(x*(a0+a1 x^2)/(x^2+b), +-1)
   with a bitcast-NOT reciprocal seed + one Newton step (max abs err 3.4e-3,
   well inside the 2e-2 gate). Units are greedily load-balanced ACT vs DVE.
"""

from contextlib import ExitStack

import numpy as np

S, A, D, H = 4096, 1024, 5, 64
NCORES = 8
ACORE = A // NCORES  # 128 atoms per core
NPAIR = ACORE // 2  # 64 pairs per core
NS = 512  # struct chunk = one PSUM bank of fp32
NCHUNK = S // NS  # 8
KG = D * 2 + 1  # 11: two atoms' descriptors + ones row for the b1 fold
NDUO = NPAIR // 2  # 32 layer-2 duos per chunk

# Layer-1 groups: trios of pairs + one solo (3*21 + 1 = 64)
L1G = [(3 * i, 3 * i + 1, 3 * i + 2) for i in range(21)] + [(63,)]
NG1 = len(L1G)  # 22

MM_DT = "float32r"
USE_DVE_TANH = True

# tanh-approx constants (fit: clip(x*(a0+a1 x^2)/(x^2+b), +-1) with
# reciprocal = seed(~bits(q))*c0 refined by one Newton step, K=2.0)
TB = 2.7762781675994193
TC0 = -0.23550172590746604
TA1 = 0.12905167923036387
TA0 = 2.7539586585465297

# engine cost model for greedy ACT/DVE balancing (ns)
def _act_cost(n):
    return (n + 352) / 1.2


def _dve_cost(n):
    return 2.0 * (n / 0.96 + 150.0)


_compiled = {}
_dve_ops = None


def _register_dve_ops():
    """Register the two tanh custom-DVE ops (additive; rows 17+ are free)."""
    global _dve_ops
    if _dve_ops is not None:
        return _dve_ops
    from concourse import dve_ops
    from concourse.dve_spec import (
        Spec, Src0, Src1, C0, C1, C2, One, maxx, minn, sq, lower, AluOp, Bin,
        _has_src1,
    )
    from concourse.dve_uop import DveOpSpec

    if "TANH_RECIP_ANT" in dve_ops.CUSTOM_DVE_SPECS:
        by = {o.name: o for o in dve_ops.OPS}
        _dve_ops = (by["TANH_RECIP_ANT"], by["TANH_NUM_ANT"])
        return _dve_ops

    _q = sq(Src0) + C0
    _nq = Bin(AluOp.BITWISE_NOT, _q, _q)
    _y0 = _nq * C1
    specA = Spec(
        body=_y0 * (C2 - _q * _y0),
        reference=lambda in0, in1, c0, c1, c2: (
            lambda q: (
                lambda y0: (y0 * (np.float32(c2) - q * y0).astype(np.float32)).astype(
                    np.float32
                )
            )(((~q.view(np.int32)).view(np.float32) * np.float32(c1)).astype(np.float32))
        )((in0 * in0 + np.float32(c0)).astype(np.float32)),
    )
    _v = ((sq(Src0) * C0 + C1) * Src0) * Src1
    specB = Spec(
        body=maxx(minn(_v, One), C2),
        reference=lambda in0, in1, c0, c1, c2: np.clip(
            ((in0 * in0 * np.float32(c0) + np.float32(c1)) * in0 * in1), c2, 1.0
        ).astype(np.float32),
    )

    made = []
    for name, spec in [("TANH_RECIP_ANT", specA), ("TANH_NUM_ANT", specB)]:
        row = dve_ops._CUSTOM_DVE_ROW_BASE + len(dve_ops.OPS)
        uops = lower(spec, ver="v3")
        assert len(uops) == 1, f"{name}: expected 1 uop, got {len(uops)}"
        sha = DveOpSpec(name=name, opcode=row, uops=uops, rd1_en=_has_src1(spec)).sha(
            "v3"
        )
        op = dve_ops.DveOp(name, spec, False, {"v3": sha})
        dve_ops.OPS.append(op)
        dve_ops.CUSTOM_DVE_SPECS[name] = spec
        dve_ops._SUB_OPCODE_FOR_NAME[name] = row
        made.append(op)
    _dve_ops = tuple(made)
    return _dve_ops


def _build(with_b2):
    import concourse.tile as tile
    import concourse.mybir as mybir
    from concourse import bacc

    dt = mybir.dt
    mdt = getattr(dt, MM_DT)
    Tanh = mybir.ActivationFunctionType.Tanh
    if USE_DVE_TANH:
        OPA, OPB = _register_dve_ops()

    nc = bacc.Bacc(
        "TRN2", target_bir_lowering=False, debug=False, num_devices=NCORES
    )
    gq = nc.declare_dram_parameter("gq", [NCHUNK, NPAIR, 32, NS], mdt, isOutput=False)
    w1 = nc.declare_dram_parameter("w1", [128, NG1 * 128], mdt, isOutput=False)
    w2 = nc.declare_dram_parameter("w2", [128, NPAIR * 128], mdt, isOutput=False)
    w3 = nc.declare_dram_parameter("w3", [128, NPAIR * 128], mdt, isOutput=False)
    if with_b2:
        b2d = nc.declare_dram_parameter("b2d", [128, NPAIR], dt.float32, isOutput=False)
    b3d = nc.declare_dram_parameter("b3d", [128, 1], dt.float32, isOutput=False)
    eo = nc.declare_dram_parameter("eo", [128, S], dt.float32, isOutput=True)

    with tile.TileContext(nc) as tc, ExitStack() as ctx:
        wp = ctx.enter_context(tc.tile_pool(name="wp", bufs=1))
        gp = ctx.enter_context(tc.tile_pool(name="gp", bufs=22))
        h1p = ctx.enter_context(tc.tile_pool(name="h1p", bufs=6))
        h2p = ctx.enter_context(tc.tile_pool(name="h2p", bufs=7))
        yp = ctx.enter_context(tc.tile_pool(name="yp", bufs=2))
        eop = ctx.enter_context(tc.tile_pool(name="eop", bufs=2))
        z1p = ctx.enter_context(tc.tile_pool(name="z1p", bufs=1, space="PSUM"))
        z2p = ctx.enter_context(tc.tile_pool(name="z2p", bufs=2, space="PSUM"))
        etp = ctx.enter_context(tc.tile_pool(name="etp", bufs=1, space="PSUM"))

        w1t = wp.tile([128, NG1 * 128], mdt)
        nc.sync.dma_start(w1t[:], w1[:])
        w2t = wp.tile([128, NPAIR * 128], mdt)
        nc.gpsimd.dma_start(w2t[:], w2[:])
        w3t = wp.tile([128, NPAIR * 128], mdt)
        nc.gpsimd.dma_start(w3t[:], w3[:])
        b3t = wp.tile([128, 1], dt.float32)
        nc.sync.dma_start(b3t[:], b3d[:])
        if with_b2:
            b2t = wp.tile([128, NPAIR], dt.float32)
            nc.sync.dma_start(b2t[:], b2d[:])

        # greedy ACT/DVE balance state
        eng_ns = {"act": 0.0, "dve": 0.0}

        def tanh_unit(h_ap, z_ap, n, dve_penalty=0.0):
            """Emit one tanh over [128, n] from PSUM z to SBUF h."""
            if USE_DVE_TANH:
                ca, cd = _act_cost(n), _dve_cost(n)
                # t1 units (dve_penalty<0) always go to ACT: they gate the
                # single-buffered z1 chain; t2 units balance 1:1
                if dve_penalty < 0 or eng_ns["act"] + ca <= eng_ns["dve"] + cd:
                    eng_ns["act"] += ca
                    nc.scalar.activation(h_ap, z_ap, Tanh)
                else:
                    eng_ns["dve"] += cd
                    yt = yp.tile([128, 3 * NS], dt.float32, name="yt", tag="yt")
                    nc.vector._custom_dve(
                        OPA, out=yt[:, :n], in0=z_ap, s0=TB, s1=TC0, imm2=2.0
                    )
                    nc.vector._custom_dve(
                        OPB, out=h_ap, in0=z_ap, in1=yt[:, :n],
                        s0=TA1, s1=TA0, imm2=-1.0,
                    )
            else:
                nc.scalar.activation(h_ap, z_ap, Tanh)

        # g staging: one DMA per (chunk, group) into a [128, 512] tile with
        # pair j of the group at partition rows 32j..32j+10.
        gstage = {}

        def ensure_g(idx):
            if idx in gstage or idx >= NCHUNK * NG1:
                return
            c, g = divmod(idx, NG1)
            pairs = L1G[g]
            gt = gp.tile([128, NS], mdt, name=f"gt{idx}", tag="gt")
            p0 = pairs[0]
            src = gq[c, p0 : p0 + len(pairs), :, :]
            q = nc.sync if idx % 2 == 0 else nc.gpsimd
            q.dma_start(gt[0 : 32 * len(pairs), :], src)
            gstage[idx] = gt

        zwu = z1p.tile([128, 3 * NS], dt.float32, name="zwu", tag="z1")
        for _ in range(16):
            nc.tensor.matmul(
                zwu[:, :NS], w2t[:, 0:128], w2t[:, 1024:1536], start=True, stop=True
            )

        h1_tiles = {}
        et_tiles = {}

        def front(c, g):
            """L1 matmuls (row-tiled trio) + tanh1."""
            idx = c * NG1 + g
            ensure_g(idx)
            for k in range(1, 5):
                ensure_g(idx + k)
            gt = gstage.pop(idx)
            pairs = L1G[g]
            n = len(pairs) * NS
            z1 = z1p.tile([128, 3 * NS], dt.float32, name="z1", tag="z1")
            for j, p in enumerate(pairs):
                nc.tensor.matmul(
                    z1[:, j * NS : (j + 1) * NS],
                    w1t[32 * j : 32 * j + KG, g * 128 : (g + 1) * 128],
                    gt[32 * j : 32 * j + KG, :],
                    start=True,
                    stop=True,
                    tile_position=(32 * j, 0),
                )
            h1 = h1p.tile([128, 3 * NS], mdt, name="h1", tag="h1")
            # tanh1 gates the single-buffered z1 chain: bias it toward ACT
            # (lower latency than the 2-pass DVE path)
            tanh_unit(h1[:, :n], z1[:, :n], n, dve_penalty=-1.0)
            h1_tiles[(c, g)] = h1

        def h1_ap(c, p):
            g, j = divmod(p, 3) if p < 63 else (21, 0)
            t = h1_tiles[(c, g)]
            return t[:, j * NS : (j + 1) * NS], (c, g)

        h2_tiles = {}

        def back_front(c, d):
            """L2 duo + tanh2."""
            z2 = z2p.tile([128, 2 * NS], dt.float32, name="z2", tag="z2")
            for i in range(2):
                p = 2 * d + i
                rhs, _ = h1_ap(c, p)
                nc.tensor.matmul(
                    z2[:, i * NS : (i + 1) * NS],
                    w2t[:, p * 128 : (p + 1) * 128],
                    rhs,
                    start=True,
                    stop=True,
                )
            if with_b2:
                for i in range(2):
                    p = 2 * d + i
                    nc.vector.tensor_scalar_add(
                        z2[:, i * NS : (i + 1) * NS],
                        z2[:, i * NS : (i + 1) * NS],
                        b2t[:, p : p + 1],
                    )
            h2 = h2p.tile([128, 2 * NS], mdt, name="h2", tag="h2")
            tanh_unit(h2[:], z2[:], 2 * NS)
            h2_tiles[(c, d)] = h2

        def back_tail(c, d):
            """L3 accumulation (lags back_front by one duo) + chunk flush."""
            if c not in et_tiles:
                et_tiles[c] = etp.tile([128, NS], dt.float32, name=f"et{c}", tag="et")
            et = et_tiles[c]
            h2 = h2_tiles.pop((c, d))
            for i in range(2):
                p = 2 * d + i
                nc.tensor.matmul(
                    et[:],
                    w3t[:, p * 128 : (p + 1) * 128],
                    h2[:, i * NS : (i + 1) * NS],
                    start=(p == 0),
                    stop=(p == NPAIR - 1),
                )
            if d == NDUO - 1:
                eot = eop.tile([128, NS], dt.float32)
                nc.vector.tensor_scalar_add(eot[:], et[:], b3t[:])
                eng_ns["dve"] += _dve_cost(NS) / 2
                nc.gpsimd.dma_start(eo[:, c * NS : (c + 1) * NS], eot[:])
                del et_tiles[c]

        # software pipeline: fronts drive; back_front trails fronts by >= 1 L1
        # group; back_tail trails back_front by one duo so the PE never stalls
        # at an L3 waiting for its tanh2.
        done_front = set()
        back_cursor = {c: 0 for c in range(NCHUNK)}
        tail_queue = []

        def pump_backs(limit):
            n_emitted = 0
            for c in range(NCHUNK):
                while back_cursor[c] < NDUO and n_emitted < limit:
                    d = back_cursor[c]
                    gmax = max(min(2 * d, 63) // 3 if 2 * d < 63 else 21,
                               (2 * d + 1) // 3 if 2 * d + 1 < 63 else 21)
                    # require two groups of lookahead within chunk (or chunk done)
                    need = (c, min(gmax + 2, NG1 - 1))
                    if need not in done_front:
                        break
                    back_front(c, d)
                    tail_queue.append((c, d))
                    if len(tail_queue) > 2:
                        back_tail(*tail_queue.pop(0))
                    back_cursor[c] = d + 1
                    n_emitted += 1
            return n_emitted

        for c in range(NCHUNK):
            for g in range(NG1):
                front(c, g)
                done_front.add((c, g))
                pump_backs(2)
        while pump_backs(NDUO * NCHUNK):
            pass
        while tail_queue:
            back_tail(*tail_queue.pop(0))
    nc.compile()
    return nc


def _prep_core(c, g, W1, b1, W2, b2, W3, b3, with_b2):
    import ml_dtypes

    at = slice(c * ACORE, (c + 1) * ACORE)
    f32 = np.float32
    mdt = ml_dtypes.bfloat16 if MM_DT == "bfloat16" else np.float32

    # gq: [NCHUNK, NPAIR, 32, NS]; per pair: rows 0-4 even atom's descriptors,
    # 5-9 odd atom's, 10 = ones (streams the b1 fold), 11-31 zero padding so
    # each trio loads as one contiguous [96, NS] DMA.
    gc = g[:, at, :]  # [S, 128, 5]
    gT = np.ascontiguousarray(gc.transpose(1, 2, 0))  # [128, 5, S]
    garr = np.zeros((NPAIR, 32, S), dtype=f32)
    garr[:, :D] = gT[0::2]
    garr[:, D : 2 * D] = gT[1::2]
    garr[:, 2 * D] = 1.0
    # [64, 32, S] -> [NCHUNK, 64, 32, NS]
    gqa = np.ascontiguousarray(
        garr.reshape(NPAIR, 32, NCHUNK, NS).transpose(2, 0, 1, 3)
    )

    W1c, b1c = W1[at], b1[at]  # [128, 5, 64], [128, 64]
    w1a = np.zeros((128, NG1 * 128), dtype=f32)
    for gi, pairs in enumerate(L1G):
        for j, p in enumerate(pairs):
            r = 32 * j
            cbase = gi * 128
            w1a[r : r + D, cbase : cbase + H] = W1c[2 * p]
            w1a[r + D : r + 2 * D, cbase + H : cbase + 128] = W1c[2 * p + 1]
            w1a[r + 2 * D, cbase : cbase + H] = b1c[2 * p]
            w1a[r + 2 * D, cbase + H : cbase + 128] = b1c[2 * p + 1]

    W2c = W2[at]  # [128, 64, 64]
    w2a = np.zeros((NPAIR, 128, 128), dtype=f32)
    w2a[:, :H, :H] = W2c[0::2]
    w2a[:, H:, H:] = W2c[1::2]
    w2d = np.ascontiguousarray(w2a.transpose(1, 0, 2)).reshape(128, NPAIR * 128)

    W3c = W3[at][..., 0]  # [128, 64]
    w3a = np.zeros((NPAIR, 128, 128), dtype=f32)
    for p in range(NPAIR):
        w3a[p, :H, 2 * p] = W3c[2 * p]
        w3a[p, H:, 2 * p + 1] = W3c[2 * p + 1]
    w3d = np.ascontiguousarray(w3a.transpose(1, 0, 2)).reshape(128, NPAIR * 128)

    b3row = np.ascontiguousarray(b3[at]).astype(f32)  # [128, 1]

    in_map = {
        "gq": gqa.astype(mdt),
        "w1": w1a.astype(mdt),
        "w2": w2d.astype(mdt),
        "w3": w3d.astype(mdt),
        "b3d": b3row,
    }
    if with_b2:
        b2c = b2[at]  # [128, 64]
        in_map["b2d"] = np.ascontiguousarray(
            np.concatenate([b2c[0::2].T, b2c[1::2].T], axis=0)
        ).astype(f32)
    return in_map


def kernel(g, W1, b1, W2, b2, W3, b3):
    from concourse.bass_utils import run_bass_kernel_spmd

    g = np.asarray(g, dtype=np.float32)
    W1 = np.asarray(W1, dtype=np.float32)
    b1 = np.asarray(b1, dtype=np.float32)
    W2 = np.asarray(W2, dtype=np.float32)
    b2 = np.asarray(b2, dtype=np.float32)
    W3 = np.asarray(W3, dtype=np.float32)
    b3 = np.asarray(b3, dtype=np.float32)

    with_b2 = bool(np.any(b2))
    if with_b2 not in _compiled:
        _compiled[with_b2] = _build(with_b2)
    nc = _compiled[with_b2]

    in_maps = [
        _prep_core(c, g, W1, b1, W2, b2, W3, b3, with_b2) for c in range(NCORES)
    ]
    res = run_bass_kernel_spmd(nc, in_maps, list(range(NCORES)))

    e = np.empty((S, A), dtype=np.float32)
    for c in range(NCORES):
        e[:, c * ACORE : (c + 1) * ACORE] = res.results[c]["eo"].T
    return e


# revision 18
# speedup vs baseline: 1.2054x; 1.2054x over previous
"""Trainium2 Bass kernel for grouped per-atom MLPs (AtomicNN energy eval).

Math: e[s, a] = W3[a].T tanh(W2[a].T tanh(W1[a].T g[s,a] + b1[a]) + b2[a]) + b3[a]
Shapes: g [4096, 1024, 5], per-atom MLP 5 -> 64 -> 64 -> 1.

Strategy (8 NeuronCores, SPMD, atom-sharded: core c owns atoms [128c, 128c+128)):
 - Atoms processed in pairs (2x64 = 128 PE rows/cols); 4096 structs stream
   through in 8 chunks of NS=512 (one PSUM bank).
 - Layer 1 (K=11 incl. a ones-row folding b1): pairs grouped in TRIOS and
   row-tiled via tile_position=(32j, 0) -- 3 concurrent matmuls in different
   32-row groups of the PE array (~3x layer-1 throughput).
 - Layer 2: full-array matmuls (K=128, M=128), pairs processed in duos so a
   single [128, 1024] PSUM tile feeds one batched tanh.
 - Layer 3: 64 accumulating matmuls per chunk into a single PSUM bank:
   lhsT[:, 2p] = [W3[2p]; 0], lhsT[:, 2p+1] = [0; W3[2p+1]] builds the
   [128 atoms, 512 structs] transposed-output block directly.
 - PSUM budget: z1 trio (3 banks) + z2 duo x2 (4) + et (1) = 8 banks exactly.
 - The PE runs throttled (1.2 GHz) unless an activity window is gap-free;
   with tanh offloaded the PE stream is the dense bottleneck and the HAM
   un-throttles for long stretches.
 - tanh is the critical path (ScalarE is the only tanh engine at 1 elem/
   lane/cycle): ~30% of tanh units are offloaded to the Vector engine via two
   registered custom-DVE ops implementing clip(x*(a0+a1 x^2)/(x^2+b), +-1)
   with a bitcast-NOT reciprocal seed + one Newton step (max abs err 3.4e-3,
   well inside the 2e-2 gate). Units are greedily load-balanced ACT vs DVE.
"""

from contextlib import ExitStack

import numpy as np

S, A, D, H = 4096, 1024, 5, 64
NCORES = 8
ACORE = A // NCORES  # 128 atoms per core
NPAIR = ACORE // 2  # 64 pairs per core
NS = 512  # struct chunk = one PSUM bank of fp32
NCHUNK = S // NS  # 8
KG = D * 2 + 1  # 11: two atoms' descriptors + ones row for the b1 fold
NDUO = NPAIR // 2  # 32 layer-2 duos per chunk

# Layer-1 groups: trios of pairs + one solo (3*21 + 1 = 64)
L1G = [(3 * i, 3 * i + 1, 3 * i + 2) for i in range(21)] + [(63,)]
NG1 = len(L1G)  # 22

MM_DT = "float32r"
USE_DVE_TANH = True

# tanh-approx constants (fit: clip(x*(a0+a1 x^2)/(x^2+b), +-1) with
# reciprocal = seed(~bits(q))*c0 refined by one Newton step, K=2.0)
TB = 2.7762781675994193
TC0 = -0.23550172590746604
TA1 = 0.12905167923036387
TA0 = 2.7539586585465297

# engine cost model for greedy ACT/DVE balancing (ns)
def _act_cost(n):
    return (n + 352) / 1.2


def _dve_cost(n):
    return 2.0 * (n / 0.96 + 150.0)


_compiled = {}
_dve_ops = None


def _register_dve_ops():
    """Register the two tanh custom-DVE ops (additive; rows 17+ are free)."""
    global _dve_ops
    if _dve_ops is not None:
        return _dve_ops
    from concourse import dve_ops
    from concourse.dve_spec import (
        Spec, Src0, Src1, C0, C1, C2, One, maxx, minn, sq, lower, AluOp, Bin,
        _has_src1,
    )
    from concourse.dve_uop import DveOpSpec

    if "TANH_RECIP_ANT" in dve_ops.CUSTOM_DVE_SPECS:
        by = {o.name: o for o in dve_ops.OPS}
        _dve_ops = (by["TANH_RECIP_ANT"], by["TANH_NUM_ANT"])
        return _dve_ops

    _q = sq(Src0) + C0
    _nq = Bin(AluOp.BITWISE_NOT, _q, _q)
    _y0 = _nq * C1
    specA = Spec(
        body=_y0 * (C2 - _q * _y0),
        reference=lambda in0, in1, c0, c1, c2: (
            lambda q: (
                lambda y0: (y0 * (np.float32(c2) - q * y0).astype(np.float32)).astype(
                    np.float32
                )
            )(((~q.view(np.int32)).view(np.float32) * np.float32(c1)).astype(np.float32))
        )((in0 * in0 + np.float32(c0)).astype(np.float32)),
    )
    _v = ((sq(Src0) * C0 + C1) * Src0) * Src1
    specB = Spec(
        body=maxx(minn(_v, One), C2),
        reference=lambda in0, in1, c0, c1, c2: np.clip(
            ((in0 * in0 * np.float32(c0) + np.float32(c1)) * in0 * in1), c2, 1.0
        ).astype(np.float32),
    )

    made = []
    for name, spec in [("TANH_RECIP_ANT", specA), ("TANH_NUM_ANT", specB)]:
        row = dve_ops._CUSTOM_DVE_ROW_BASE + len(dve_ops.OPS)
        uops = lower(spec, ver="v3")
        assert len(uops) == 1, f"{name}: expected 1 uop, got {len(uops)}"
        sha = DveOpSpec(name=name, opcode=row, uops=uops, rd1_en=_has_src1(spec)).sha(
            "v3"
        )
        op = dve_ops.DveOp(name, spec, False, {"v3": sha})
        dve_ops.OPS.append(op)
        dve_ops.CUSTOM_DVE_SPECS[name] = spec
        dve_ops._SUB_OPCODE_FOR_NAME[name] = row
        made.append(op)
    _dve_ops = tuple(made)
    return _dve_ops


def _build(with_b2):
    import concourse.tile as tile
    import concourse.mybir as mybir
    from concourse import bacc

    dt = mybir.dt
    mdt = getattr(dt, MM_DT)
    Tanh = mybir.ActivationFunctionType.Tanh
    if USE_DVE_TANH:
        OPA, OPB = _register_dve_ops()

    nc = bacc.Bacc(
        "TRN2", target_bir_lowering=False, debug=False, num_devices=NCORES
    )
    gq = nc.declare_dram_parameter("gq", [NCHUNK, NPAIR, 32, NS], mdt, isOutput=False)
    w1 = nc.declare_dram_parameter("w1", [128, NG1 * 128], mdt, isOutput=False)
    w2 = nc.declare_dram_parameter("w2", [128, NPAIR * 128], mdt, isOutput=False)
    w3 = nc.declare_dram_parameter("w3", [128, NPAIR * 128], mdt, isOutput=False)
    if with_b2:
        b2d = nc.declare_dram_parameter("b2d", [128, NPAIR], dt.float32, isOutput=False)
    b3d = nc.declare_dram_parameter("b3d", [128, 1], dt.float32, isOutput=False)
    eo = nc.declare_dram_parameter("eo", [128, S], dt.float32, isOutput=True)

    with tile.TileContext(nc) as tc, ExitStack() as ctx:
        wp = ctx.enter_context(tc.tile_pool(name="wp", bufs=1))
        gp = ctx.enter_context(tc.tile_pool(name="gp", bufs=22))
        h1p = ctx.enter_context(tc.tile_pool(name="h1p", bufs=6))
        h2p = ctx.enter_context(tc.tile_pool(name="h2p", bufs=6))
        yp = ctx.enter_context(tc.tile_pool(name="yp", bufs=2))
        eop = ctx.enter_context(tc.tile_pool(name="eop", bufs=2))
        z1p = ctx.enter_context(tc.tile_pool(name="z1p", bufs=1, space="PSUM"))
        z2p = ctx.enter_context(tc.tile_pool(name="z2p", bufs=2, space="PSUM"))
        etp = ctx.enter_context(tc.tile_pool(name="etp", bufs=1, space="PSUM"))

        w1t = wp.tile([128, NG1 * 128], mdt)
        nc.sync.dma_start(w1t[:], w1[:])
        w2t = wp.tile([128, NPAIR * 128], mdt)
        nc.gpsimd.dma_start(w2t[:], w2[:])
        w3t = wp.tile([128, NPAIR * 128], mdt)
        nc.gpsimd.dma_start(w3t[:], w3[:])
        b3t = wp.tile([128, 1], dt.float32)
        nc.sync.dma_start(b3t[:], b3d[:])
        if with_b2:
            b2t = wp.tile([128, NPAIR], dt.float32)
            nc.sync.dma_start(b2t[:], b2d[:])

        # greedy ACT/DVE balance state
        eng_ns = {"act": 0.0, "dve": 0.0}

        def tanh_unit(h_ap, z_ap, n, dve_penalty=0.0):
            """Emit one tanh over [128, n] from PSUM z to SBUF h."""
            if USE_DVE_TANH:
                ca, cd = _act_cost(n), _dve_cost(n)
                # prefer ACT (single-instruction latency); DVE is overflow
                if eng_ns["act"] + ca <= 1.8 * (eng_ns["dve"] + cd) + dve_penalty:
                    eng_ns["act"] += ca
                    nc.scalar.activation(h_ap, z_ap, Tanh)
                else:
                    eng_ns["dve"] += cd
                    yt = yp.tile([128, 3 * NS], dt.float32, name="yt", tag="yt")
                    nc.vector._custom_dve(
                        OPA, out=yt[:, :n], in0=z_ap, s0=TB, s1=TC0, imm2=2.0
                    )
                    nc.vector._custom_dve(
                        OPB, out=h_ap, in0=z_ap, in1=yt[:, :n],
                        s0=TA1, s1=TA0, imm2=-1.0,
                    )
            else:
                nc.scalar.activation(h_ap, z_ap, Tanh)

        # g staging: one DMA per (chunk, group) into a [128, 512] tile with
        # pair j of the group at partition rows 32j..32j+10.
        gstage = {}

        def ensure_g(idx):
            if idx in gstage or idx >= NCHUNK * NG1:
                return
            c, g = divmod(idx, NG1)
            pairs = L1G[g]
            gt = gp.tile([128, NS], mdt, name=f"gt{idx}", tag="gt")
            p0 = pairs[0]
            src = gq[c, p0 : p0 + len(pairs), :, :]
            q = nc.sync if idx % 2 == 0 else nc.gpsimd
            q.dma_start(gt[0 : 32 * len(pairs), :], src)
            gstage[idx] = gt

        zwu = z1p.tile([128, 3 * NS], dt.float32, name="zwu", tag="z1")
        for _ in range(16):
            nc.tensor.matmul(
                zwu[:, :NS], w2t[:, 0:128], w2t[:, 1024:1536], start=True, stop=True
            )

        h1_tiles = {}
        et_tiles = {}

        def front(c, g):
            """L1 matmuls (row-tiled trio) + tanh1."""
            idx = c * NG1 + g
            ensure_g(idx)
            for k in range(1, 5):
                ensure_g(idx + k)
            gt = gstage.pop(idx)
            pairs = L1G[g]
            n = len(pairs) * NS
            z1 = z1p.tile([128, 3 * NS], dt.float32, name="z1", tag="z1")
            for j, p in enumerate(pairs):
                nc.tensor.matmul(
                    z1[:, j * NS : (j + 1) * NS],
                    w1t[32 * j : 32 * j + KG, g * 128 : (g + 1) * 128],
                    gt[32 * j : 32 * j + KG, :],
                    start=True,
                    stop=True,
                    tile_position=(32 * j, 0),
                )
            h1 = h1p.tile([128, 3 * NS], mdt, name="h1", tag="h1")
            # tanh1 gates the single-buffered z1 chain: bias it toward ACT
            # (lower latency than the 2-pass DVE path)
            tanh_unit(h1[:, :n], z1[:, :n], n, dve_penalty=900.0)
            h1_tiles[(c, g)] = h1

        def h1_ap(c, p):
            g, j = divmod(p, 3) if p < 63 else (21, 0)
            t = h1_tiles[(c, g)]
            return t[:, j * NS : (j + 1) * NS], (c, g)

        h2_tiles = {}

        def back_front(c, d):
            """L2 duo + tanh2."""
            z2 = z2p.tile([128, 2 * NS], dt.float32, name="z2", tag="z2")
            for i in range(2):
                p = 2 * d + i
                rhs, _ = h1_ap(c, p)
                nc.tensor.matmul(
                    z2[:, i * NS : (i + 1) * NS],
                    w2t[:, p * 128 : (p + 1) * 128],
                    rhs,
                    start=True,
                    stop=True,
                )
            if with_b2:
                for i in range(2):
                    p = 2 * d + i
                    nc.vector.tensor_scalar_add(
                        z2[:, i * NS : (i + 1) * NS],
                        z2[:, i * NS : (i + 1) * NS],
                        b2t[:, p : p + 1],
                    )
            h2 = h2p.tile([128, 2 * NS], mdt, name="h2", tag="h2")
            tanh_unit(h2[:], z2[:], 2 * NS)
            h2_tiles[(c, d)] = h2

        def back_tail(c, d):
            """L3 accumulation (lags back_front by one duo) + chunk flush."""
            if c not in et_tiles:
                et_tiles[c] = etp.tile([128, NS], dt.float32, name=f"et{c}", tag="et")
            et = et_tiles[c]
            h2 = h2_tiles.pop((c, d))
            for i in range(2):
                p = 2 * d + i
                nc.tensor.matmul(
                    et[:],
                    w3t[:, p * 128 : (p + 1) * 128],
                    h2[:, i * NS : (i + 1) * NS],
                    start=(p == 0),
                    stop=(p == NPAIR - 1),
                )
            if d == NDUO - 1:
                eot = eop.tile([128, NS], dt.float32)
                nc.vector.tensor_scalar_add(eot[:], et[:], b3t[:])
                eng_ns["dve"] += _dve_cost(NS) / 2
                nc.gpsimd.dma_start(eo[:, c * NS : (c + 1) * NS], eot[:])
                del et_tiles[c]

        # software pipeline: fronts drive; back_front trails fronts by >= 1 L1
        # group; back_tail trails back_front by one duo so the PE never stalls
        # at an L3 waiting for its tanh2.
        done_front = set()
        back_cursor = {c: 0 for c in range(NCHUNK)}
        tail_queue = []

        def pump_backs(limit):
            n_emitted = 0
            for c in range(NCHUNK):
                while back_cursor[c] < NDUO and n_emitted < limit:
                    d = back_cursor[c]
                    gmax = max(min(2 * d, 63) // 3 if 2 * d < 63 else 21,
                               (2 * d + 1) // 3 if 2 * d + 1 < 63 else 21)
                    # require two groups of lookahead within chunk (or chunk done)
                    need = (c, min(gmax + 2, NG1 - 1))
                    if need not in done_front:
                        break
                    back_front(c, d)
                    tail_queue.append((c, d))
                    if len(tail_queue) > 2:
                        back_tail(*tail_queue.pop(0))
                    back_cursor[c] = d + 1
                    n_emitted += 1
            return n_emitted

        for c in range(NCHUNK):
            for g in range(NG1):
                front(c, g)
                done_front.add((c, g))
                pump_backs(2)
        while pump_backs(NDUO * NCHUNK):
            pass
        while tail_queue:
            back_tail(*tail_queue.pop(0))
    nc.compile()
    return nc


def _prep_core(c, g, W1, b1, W2, b2, W3, b3, with_b2):
    import ml_dtypes

    at = slice(c * ACORE, (c + 1) * ACORE)
    f32 = np.float32
    mdt = ml_dtypes.bfloat16 if MM_DT == "bfloat16" else np.float32

    # gq: [NCHUNK, NPAIR, 32, NS]; per pair: rows 0-4 even atom's descriptors,
    # 5-9 odd atom's, 10 = ones (streams the b1 fold), 11-31 zero padding so
    # each trio loads as one contiguous [96, NS] DMA.
    gc = g[:, at, :]  # [S, 128, 5]
    gT = np.ascontiguousarray(gc.transpose(1, 2, 0))  # [128, 5, S]
    garr = np.zeros((NPAIR, 32, S), dtype=f32)
    garr[:, :D] = gT[0::2]
    garr[:, D : 2 * D] = gT[1::2]
    garr[:, 2 * D] = 1.0
    # [64, 32, S] -> [NCHUNK, 64, 32, NS]
    gqa = np.ascontiguousarray(
        garr.reshape(NPAIR, 32, NCHUNK, NS).transpose(2, 0, 1, 3)
    )

    W1c, b1c = W1[at], b1[at]  # [128, 5, 64], [128, 64]
    w1a = np.zeros((128, NG1 * 128), dtype=f32)
    for gi, pairs in enumerate(L1G):
        for j, p in enumerate(pairs):
            r = 32 * j
            cbase = gi * 128
            w1a[r : r + D, cbase : cbase + H] = W1c[2 * p]
            w1a[r + D : r + 2 * D, cbase + H : cbase + 128] = W1c[2 * p + 1]
            w1a[r + 2 * D, cbase : cbase + H] = b1c[2 * p]
            w1a[r + 2 * D, cbase + H : cbase + 128] = b1c[2 * p + 1]

    W2c = W2[at]  # [128, 64, 64]
    w2a = np.zeros((NPAIR, 128, 128), dtype=f32)
    w2a[:, :H, :H] = W2c[0::2]
    w2a[:, H:, H:] = W2c[1::2]
    w2d = np.ascontiguousarray(w2a.transpose(1, 0, 2)).reshape(128, NPAIR * 128)

    W3c = W3[at][..., 0]  # [128, 64]
    w3a = np.zeros((NPAIR, 128, 128), dtype=f32)
    for p in range(NPAIR):
        w3a[p, :H, 2 * p] = W3c[2 * p]
        w3a[p, H:, 2 * p + 1] = W3c[2 * p + 1]
    w3d = np.ascontiguousarray(w3a.transpose(1, 0, 2)).reshape(128, NPAIR * 128)

    b3row = np.ascontiguousarray(b3[at]).astype(f32)  # [128, 1]

    in_map = {
        "gq": gqa.astype(mdt),
        "w1": w1a.astype(mdt),
        "w2": w2d.astype(mdt),
        "w3": w3d.astype(mdt),
        "b3d": b3row,
    }
    if with_b2:
        b2c = b2[at]  # [128, 64]
        in_map["b2d"] = np.ascontiguousarray(
            np.concatenate([b2c[0::2].T, b2c[1::2].T], axis=0)
        ).astype(f32)
    return in_map


def kernel(g, W1, b1, W2, b2, W3, b3):
    from concourse.bass_utils import run_bass_kernel_spmd

    g = np.asarray(g, dtype=np.float32)
    W1 = np.asarray(W1, dtype=np.float32)
    b1 = np.asarray(b1, dtype=np.float32)
    W2 = np.asarray(W2, dtype=np.float32)
    b2 = np.asarray(b2, dtype=np.float32)
    W3 = np.asarray(W3, dtype=np.float32)
    b3 = np.asarray(b3, dtype=np.float32)

    with_b2 = bool(np.any(b2))
    if with_b2 not in _compiled:
        _compiled[with_b2] = _build(with_b2)
    nc = _compiled[with_b2]

    in_maps = [
        _prep_core(c, g, W1, b1, W2, b2, W3, b3, with_b2) for c in range(NCORES)
    ]
    res = run_bass_kernel_spmd(nc, in_maps, list(range(NCORES)))

    e = np.empty((S, A), dtype=np.float32)
    for c in range(NCORES):
        e[:, c * ACORE : (c + 1) * ACORE] = res.results[c]["eo"].T
    return e
